# revision 18
# baseline (speedup 1.0000x reference)
"""CRF negative-log-likelihood loss (BERT_BiLSTM_CRF) on 8 TRN2 NeuronCores.

Strategy (data-parallel over batch, 64 sequences/core):
 - Linear-space forward algorithm with the 32x32 exp(transitions) matrix as
   PE matmul weights (block-diag [fwd | bwd], bf16); per step one bf16 matmul
   + one DVE elementwise multiply by exp(emissions - MU).
 - Forward and backward (meet-in-the-middle) chains run in the same per-tick
   matmul, halving the serial step count to 1024.
 - The exp(emissions) pipeline is decoupled from the scan: per tick one PE
   transpose lands in a grouped PSUM tile, one batched ACT exp per 8 ticks
   writes a bf16 xt chunk a full 64 ticks ahead of use. The scan's serial
   critical path is only matmul -> DVE multiply -> matmul.
 - Emissions stream in big per-chunk DMAs (fwd natural, bwd reversed).
 - Periodic renorm (every 128 ticks) by a proxy-row reciprocal keeps bf16
   exponents in range; log(scale) accumulates into the per-sequence offset.
 - Gold emission score via one-hot compare/multiply/reduce on DVE, split in
   small pieces interleaved between ticks so they never stall the scan.
"""
import numpy as np

TAGSET = 32
START = 30
STOP = 31
B = 512
S = 2048
NCORES = 8
BC = B // NCORES          # 64 sequences per core
HALF = S // 2             # 1024 ticks per direction
CH = 64                   # emission steps per streamed chunk
NCH = HALF // CH          # 16 chunks per direction
GRP = 8                   # ticks per transpose/exp group (one PSUM bank)
MU = np.float32(4.3226)   # mean log-growth per step (measured offline)
REN = 256                 # renorm period in ticks (bf16 exponent range is
                          # ample: drift ~sqrt(REN)*2.5 nats << 88 nats)

_CACHE = {}


def _build_nc(debug=False, gold=2, reps=1):
    import concourse.bacc as bacc
    import concourse.bass as bass
    import concourse.tile as tile
    from concourse import mybir

    f32 = mybir.dt.float32
    bf16 = mybir.dt.bfloat16
    i32 = mybir.dt.int32
    AF = mybir.ActivationFunctionType
    OP = mybir.AluOpType
    AX = mybir.AxisListType

    nc = bacc.Bacc("TRN2", target_bir_lowering=False, debug=False,
                   num_devices=NCORES)

    em_d = nc.dram_tensor("emissions", [BC, S, TAGSET], f32,
                          kind="ExternalInput").ap()
    tg_d = nc.dram_tensor("tags", [BC, S], i32, kind="ExternalInput").ap()
    tr_d = nc.dram_tensor("transitions", [TAGSET, TAGSET], f32,
                          kind="ExternalInput").ap()
    nll_d = nc.dram_tensor("nll", [1, BC], f32, kind="ExternalOutput").ap()

    with tile.TileContext(nc) as tc:
        with (
            tc.tile_pool(name="const", bufs=1) as cp,
            tc.tile_pool(name="chunk", bufs=3) as ccp,
            tc.tile_pool(name="oh", bufs=2) as ohp,
            tc.tile_pool(name="xt", bufs=2) as xtp,
            tc.tile_pool(name="state", bufs=3) as stp,
            tc.tile_pool(name="small", bufs=2) as smp,
            tc.tile_pool(name="trp", bufs=2, space="PSUM") as trp,
            tc.tile_pool(name="mmp", bufs=2, space="PSUM") as mmp,
            tc.tile_pool(name="finp", bufs=1, space="PSUM") as fip,
        ):
            # Optional on-device repetition (used only by test.py timing —
            # slope over reps cancels host/tunnel dispatch overhead).
            _loop = None
            if reps > 1:
                _loop = tc.For_i(
                    0, reps, 1,
                    hint_engines=(mybir.EngineType.PE, mybir.EngineType.DVE,
                                  mybir.EngineType.Activation))
                _loop.__enter__()

            # ---------------- setup: weights, identity, ones ----------------
            w = cp.tile([64, 64], f32)
            nc.vector.memset(w[:], 0.0)
            # fwd block: w[p, t] = trans[t, p]  (strided transpose DMA, tiny)
            nc.sync.dma_start(w[0:32, 0:32], tr_d.rearrange("a b -> b a"))
            # bwd block: w[32+p, 32+t] = trans[p, t]
            nc.sync.dma_start(w[32:64, 32:64], tr_d)
            # clamp -1e4 entries so the exp LUT stays in-range, then exp
            nc.vector.tensor_scalar_max(w[:], w[:], -80.0)
            nc.scalar.activation(w[:], w[:], AF.Exp)
            # zero the off-diagonal blocks again (exp(0)=1 crept in)
            nc.vector.memset(w[0:32, 32:64], 0.0)
            nc.vector.memset(w[32:64, 0:32], 0.0)
            # bf16 weights: bf16 matmul streams 1 col/cycle (fp32 is 4)
            wb16 = cp.tile([64, 64], bf16)
            nc.vector.tensor_copy(wb16[:], w[:])

            ones_t = cp.tile([64, 64], f32)
            nc.vector.memset(ones_t[:], 1.0)
            ones_b = cp.tile([64, 64], bf16)
            nc.vector.memset(ones_b[:], 1.0)
            negmu = cp.tile([64, 1], f32)
            nc.vector.memset(negmu[:], -float(MU))
            ident = cp.tile([64, 64], f32)
            nc.gpsimd.affine_select(
                out=ident[:], in_=ones_t[:], pattern=[[-1, 64]],
                compare_op=OP.is_equal, fill=0.0, base=0, channel_multiplier=1)

            # ---------------- gold emission score (one-hot) ----------------
            # e_score[b] = sum_s em[b, s, tags[b, s]]  computed in GRP-step
            # pieces as sum((t-iota == tag) * em) with DVE is_equal +
            # mult + reduce. The transition part of the gold score is tiny
            # and is folded in on the host during unshard.
            tags_sb = cp.tile([BC, S], i32)
            nc.sync.dma_start(tags_sb[:], tg_d)
            iota_t = cp.tile([BC, GRP * TAGSET], i32)
            nc.gpsimd.iota(iota_t[:], pattern=[[0, GRP], [1, TAGSET]], base=0,
                           channel_multiplier=0)
            NACC = NCH * (CH // GRP)  # one accum column per piece
            acc_e = cp.tile([BC, NACC], f32)
            nc.vector.memset(acc_e[:], 0.0)

            # ---------------- scan state init ----------------
            offacc = cp.tile([64, 64], f32)
            nc.vector.memset(offacc[:], 0.0)

            state = stp.tile([64, 64], bf16, tag="state")
            # one-hot inits: fwd rows = e_START, bwd rows = e_STOP
            nc.gpsimd.affine_select(
                out=state[0:32, :], in_=ones_b[0:32, :], pattern=[[0, 64]],
                compare_op=OP.is_equal, fill=0.0, base=-START,
                channel_multiplier=1)
            nc.gpsimd.affine_select(
                out=state[32:64, :], in_=ones_b[32:64, :], pattern=[[0, 64]],
                compare_op=OP.is_equal, fill=0.0, base=-STOP,
                channel_multiplier=1)

            # ---------------- chunk machinery ----------------
            # comb(g): [BC, CH*2*TAGSET] with interleaved layout per local
            # step l: cols [l*64, l*64+32) = emissions[:, g*CH+l, :] (fwd),
            # cols [l*64+32, l*64+64) = emissions[:, S-1-g*CH-l, :] (bwd).
            def load_chunk(g):
                comb = ccp.tile([BC, 2 * CH * TAGSET], f32, tag="comb")
                cv = comb[:].rearrange("b (s u t) -> b s u t",
                                       u=2, t=TAGSET)
                nc.sync.dma_start(cv[:, :, 0, :],
                                  em_d[:, g * CH:(g + 1) * CH, :])
                nc.sync.dma_start(
                    cv[:, :, 1, :],
                    em_d[:, S - 1 - g * CH:S - (g + 1) * CH - 1:-1, :])
                return comb

            # xt chunk: [64, CH*64] bf16; tick l occupies cols l*64:(l+1)*64
            # with fwd tags on partitions 0-31, bwd tags on 32-63.
            def prep_tick(comb, xt_chunk, l, trg):
                # one PE transpose per tick into the group PSUM tile
                j = l % GRP
                nc.tensor.transpose(
                    trg[:, j * 64:j * 64 + 64],
                    comb[:, l * 64:(l + 1) * 64], ident[:])
                if j == GRP - 1:
                    base = (l - j) * 64
                    nc.scalar.activation(
                        xt_chunk[:, base:base + GRP * 64], trg[:],
                        AF.Exp, bias=negmu[:])

            def gold_piece(comb, g, k):
                # one-hot gold for ticks [k*GRP, (k+1)*GRP) of chunk g,
                # both directions: 2 strided is_equal + 1 mult + 1 reduce
                iview = iota_t[:].rearrange("b (l t) -> b l t", t=TAGSET)
                oh = ohp.tile([BC, GRP * 2 * TAGSET], f32, tag="oh")
                ov = oh[:].rearrange("b (l u t) -> b l u t", u=2, t=TAGSET)
                for u in range(2):
                    if u == 0:
                        tsl = tags_sb[:, g * CH + k * GRP:
                                      g * CH + (k + 1) * GRP]
                    else:
                        hi = S - 1 - g * CH - k * GRP
                        tsl = tags_sb[:, hi:hi - GRP:-1]
                    tbc = tsl.rearrange("b l -> b l ()").to_broadcast(
                        [BC, GRP, TAGSET])
                    nc.vector.tensor_tensor(
                        out=ov[:, :, u, :], in0=iview, in1=tbc,
                        op=OP.is_equal)
                scrap = ohp.tile([BC, GRP * 2 * TAGSET], f32, tag="scrap")
                nc.vector.tensor_mul(scrap[:], oh[:],
                                     comb[:, k * GRP * 64:(k + 1) * GRP * 64])
                col = g * (CH // GRP) + k
                nc.vector.tensor_reduce(
                    acc_e[:, col:col + 1], scrap[:], axis=AX.X, op=OP.add)

            # ---------------- prologue: chunk 0 prep ----------------
            comb_cur = load_chunk(0)
            trg = None
            xtc_cur = xtp.tile([64, CH * 64], bf16, tag="xtc")
            for l in range(CH):
                if l % GRP == 0:
                    trg = trp.tile([64, GRP * 64], f32, tag="trg")
                prep_tick(comb_cur, xtc_cur, l, trg)

            # ---------------- main scan ----------------
            for g in range(NCH):
                if g + 1 < NCH:
                    comb_nxt = load_chunk(g + 1)
                    xtc_nxt = xtp.tile([64, CH * 64], bf16, tag="xtc")
                else:
                    comb_nxt = None
                    xtc_nxt = None
                for l in range(CH):
                    tau = g * CH + l
                    # pipeline: prep next chunk's xt, one tick per tick
                    if comb_nxt is not None:
                        if l % GRP == 0:
                            trg = trp.tile([64, GRP * 64], f32, tag="trg")
                        prep_tick(comb_nxt, xtc_nxt, l, trg)
                    # gold pieces for the current chunk, spread out
                    if gold >= 2 and l % GRP == 3:
                        gold_piece(comb_cur, g, l // GRP)

                    ps = mmp.tile([64, 64], f32, tag="mm")
                    nc.tensor.matmul(ps[:], wb16[:], state[:], start=True,
                                     stop=True)
                    nstate = stp.tile([64, 64], bf16, tag="state")
                    nc.vector.tensor_mul(nstate[:], ps[:],
                                         xtc_cur[:, l * 64:(l + 1) * 64])
                    state = nstate

                    if (tau + 1) % REN == 0:
                        # proxy rows to f32 first (reciprocal/Ln need f32 in)
                        pr = smp.tile([64, 64], f32, tag="pr")
                        nc.vector.tensor_copy(pr[0:1, :], state[0:1, :])
                        nc.vector.tensor_copy(pr[32:33, :], state[32:33, :])
                        rec = smp.tile([64, 64], f32, tag="rec")
                        nc.vector.reciprocal(rec[0:1, :], pr[0:1, :])
                        nc.vector.reciprocal(rec[32:33, :], pr[32:33, :])
                        bc_ps = fip.tile([64, 64], f32, tag="bc")
                        nc.tensor.matmul(bc_ps[0:32, :], ones_t[0:1, 0:32],
                                         rec[0:1, :], start=True, stop=True)
                        nc.tensor.matmul(bc_ps[32:64, :], ones_t[32:33, 0:32],
                                         rec[32:33, :], start=True, stop=True,
                                         tile_position=(32, 32))
                        lg = smp.tile([64, 64], f32, tag="lg")
                        nc.scalar.activation(lg[0:1, :], pr[0:1, :], AF.Ln)
                        nc.scalar.activation(lg[32:33, :], pr[32:33, :],
                                             AF.Ln)
                        nc.vector.tensor_add(offacc[0:1, :], offacc[0:1, :],
                                             lg[0:1, :])
                        nc.vector.tensor_add(offacc[32:33, :],
                                             offacc[32:33, :], lg[32:33, :])
                        rstate = stp.tile([64, 64], bf16, tag="state")
                        nc.vector.tensor_mul(rstate[:], state[:], bc_ps[:])
                        state = rstate
                comb_cur = comb_nxt
                xtc_cur = xtc_nxt

            # ---------------- finale ----------------
            # beta_1023 = M^T gamma_1024: bwd-final matmul with weights
            # placed so the output lands on partitions 0-31 (aligned with
            # the fwd state for the elementwise dot).
            wb = cp.tile([64, 64], bf16)
            nc.vector.memset(wb[:], 0.0)
            nc.sync.dma_start(wb[32:64, 0:32], wb16[32:64, 32:64])
            psf = mmp.tile([64, 64], f32, tag="mm")
            nc.tensor.matmul(psf[0:32, :], wb[32:64, 0:32], state[32:64, :],
                             start=True, stop=True)
            zp = smp.tile([64, 64], f32, tag="zp")
            nc.vector.tensor_mul(zp[0:32, :], psf[0:32, :], state[0:32, :])
            zsum = fip.tile([1, 64], f32, tag="zsum")
            nc.tensor.matmul(zsum[0:1, :], ones_t[0:32, 0:1], zp[0:32, :],
                             start=True, stop=True)
            gold_c = cp.tile([BC, 1], f32)
            nc.vector.tensor_reduce(gold_c[:], acc_e[:], axis=AX.X, op=OP.add)
            lz = smp.tile([64, 64], f32, tag="lz")
            nc.scalar.activation(lz[0:1, :], zsum[0:1, :], AF.Ln)
            ob = smp.tile([64, 64], f32, tag="ob")
            nc.sync.dma_start(ob[0:1, :], offacc[32:33, :])
            nc.vector.tensor_add(lz[0:1, :], lz[0:1, :], offacc[0:1, :])
            nc.vector.tensor_add(lz[0:1, :], lz[0:1, :], ob[0:1, :])
            # logZ = lz + MU*S;   nll = logZ - gold
            goldT = fip.tile([1, 64], f32, tag="goldT")
            nc.tensor.transpose(goldT[0:1, :], gold_c[:, 0:1], ident[:])
            nc.vector.tensor_sub(lz[0:1, :], lz[0:1, :], goldT[0:1, :])
            nc.vector.tensor_scalar_add(lz[0:1, :], lz[0:1, :],
                                        float(MU) * S)
            nc.sync.dma_start(nll_d, lz[0:1, :])

            if _loop is not None:
                _loop.__exit__(None, None, None)

    nc.compile()
    return nc


def _get_nc():
    if "nc" not in _CACHE:
        _CACHE["nc"] = _build_nc()
    return _CACHE["nc"]


def kernel(emissions, transitions, tags):
    from concourse.bass_utils import run_bass_kernel_spmd

    em = np.ascontiguousarray(np.asarray(emissions, dtype=np.float32))
    tr = np.ascontiguousarray(np.asarray(transitions, dtype=np.float32))
    tg = np.ascontiguousarray(np.asarray(tags, dtype=np.int32))

    nc = _get_nc()
    in_maps = [
        {
            "emissions": em[c * BC:(c + 1) * BC],
            "tags": tg[c * BC:(c + 1) * BC],
            "transitions": tr,
        }
        for c in range(NCORES)
    ]
    res = run_bass_kernel_spmd(nc, in_maps, list(range(NCORES)))
    nll = np.concatenate([res.results[c]["nll"][0] for c in range(NCORES)])
    t_sc = (tr[tg[:, 1:], tg[:, :-1]].sum(axis=1)
            + tr[tg[:, 0], START] + tr[STOP, tg[:, -1]])
    total = np.sum(nll.astype(np.float64)) - np.sum(t_sc.astype(np.float64))
    return np.array(total, dtype=np.float32)


# revision 24
# speedup vs baseline: 1.6687x; 1.6687x over previous
"""CRF negative-log-likelihood loss (BERT_BiLSTM_CRF) on 8 TRN2 NeuronCores.

v4: rank-1 segment factorization (data-parallel over batch, 64 seqs/core).

The linear-space forward recursion a_t = D_t M a_{t-1} is split into
NSEG=8 segments of L=256 steps. Products of positive matrices contract
to rank-1 (Birkhoff), so each segment's transfer matrix P_k is summarized
by probe chains computed with the SAME per-tick kernel structure:
  f_k = P_k e   (forward chain; f_1 starts from the true one-hot START)
  h_k: u <- D_t M^T u over the segment, descending   ((M^T e)^T P_k = h_k^T M)
Stitch:  logZ = log(w^T f_m) + sum_{k>=2}[log(h_k^T M f_{k-1})
                                          - log(h_k^T M e)] + offsets + MU*S
(verified exact to 1e-11 in fp64 on the real data: the rank-2 component
of a 256-step product is ~0).

All 15 chains run CONCURRENTLY: state is [64, 512] bf16 = (fwd tags |
bwd tags) x (segment, sequence), so the serial scan is 256 ticks of one
bf16 matmul [64,512] + one DVE multiply — 4x fewer serial round trips
than meet-in-the-middle. Per tick, 8 PE transposes + 1 batched ACT exp
produce the tick's xt [64,512] from chunked emission DMAs (fwd natural,
bwd reversed), a chunk ahead of use. Gold emission score via one-hot
pieces on DVE spread between ticks; renorm once at tick 127 by proxy-row
reciprocal (h-chain scale offsets cancel in the stitch, so only f-chain
logs accumulate).
"""
import numpy as np

TAGSET = 32
START = 30
STOP = 31
B = 512
S = 2048
NCORES = 8
BC = B // NCORES          # 64 sequences per core
NSEG = 8                  # segments (rank-1 factorization)
L = S // NSEG             # 256 serial ticks
CH = 16                   # ticks per emission chunk
NCHK = L // CH            # 16 chunks
MU = np.float32(4.3226)   # mean log-growth per step (measured offline)

_CACHE = {}


def _build_nc(debug=False, gold=2, reps=1):
    import concourse.bacc as bacc
    import concourse.bass as bass
    import concourse.tile as tile
    from concourse import mybir

    f32 = mybir.dt.float32
    bf16 = mybir.dt.bfloat16
    i32 = mybir.dt.int32
    AF = mybir.ActivationFunctionType
    OP = mybir.AluOpType
    AX = mybir.AxisListType

    nc = bacc.Bacc("TRN2", target_bir_lowering=False, debug=False,
                   num_devices=NCORES)

    em_d = nc.dram_tensor("emissions", [BC, S, TAGSET], f32,
                          kind="ExternalInput").ap()
    tg_d = nc.dram_tensor("tags", [BC, S], i32, kind="ExternalInput").ap()
    tr_d = nc.dram_tensor("transitions", [TAGSET, TAGSET], f32,
                          kind="ExternalInput").ap()
    nll_d = nc.dram_tensor("nll", [1, BC], f32, kind="ExternalOutput").ap()

    NC = NSEG * 64            # state columns
    # emissions viewed [b, (seg, step), tag]
    em_r = em_d.rearrange("b (j s) t -> b s j t", j=NSEG)

    with tile.TileContext(nc) as tc:
        with (
            tc.tile_pool(name="const", bufs=1) as cp,
            tc.tile_pool(name="chunk", bufs=2) as ccp,
            tc.tile_pool(name="oh", bufs=2) as ohp,
            tc.tile_pool(name="xt", bufs=CH + 3) as xtp,
            tc.tile_pool(name="state", bufs=3) as stp,
            tc.tile_pool(name="small", bufs=2) as smp,
            tc.tile_pool(name="trp", bufs=3, space="PSUM") as trp,
            tc.tile_pool(name="mmp", bufs=2, space="PSUM") as mmp,
            tc.tile_pool(name="finp", bufs=1, space="PSUM") as fip,
        ):
            _loop = None
            if reps > 1:
                _loop = tc.For_i(
                    0, reps, 1,
                    hint_engines=(mybir.EngineType.PE, mybir.EngineType.DVE,
                                  mybir.EngineType.Activation))
                _loop.__enter__()

            # ---------------- setup: weights, identity, ones ----------------
            w = cp.tile([64, 64], f32)
            nc.vector.memset(w[:], 0.0)
            # fwd block: w[p, t] = trans[t, p] -> applies M = exp(trans)
            nc.sync.dma_start(w[0:32, 0:32], tr_d.rearrange("a b -> b a"))
            # bwd block: w[32+p, 32+t] = trans[p, t] -> applies M^T
            nc.sync.dma_start(w[32:64, 32:64], tr_d)
            nc.vector.tensor_scalar_max(w[:], w[:], -80.0)
            nc.scalar.activation(w[:], w[:], AF.Exp)
            nc.vector.memset(w[0:32, 32:64], 0.0)
            nc.vector.memset(w[32:64, 0:32], 0.0)
            wb16 = cp.tile([64, 64], bf16)
            nc.vector.tensor_copy(wb16[:], w[:])

            ones_t = cp.tile([64, 64], f32)
            nc.vector.memset(ones_t[:], 1.0)
            negmu = cp.tile([64, 1], f32)
            nc.vector.memset(negmu[:], -float(MU))
            ident = cp.tile([64, 64], f32)
            nc.gpsimd.affine_select(
                out=ident[:], in_=ones_t[:], pattern=[[-1, 64]],
                compare_op=OP.is_equal, fill=0.0, base=0, channel_multiplier=1)

            # stitch constants: c = M e (bwd-block row sums), w_stop vector
            cvec = cp.tile([64, 1], f32)
            nc.vector.tensor_reduce(cvec[32:64, :], w[32:64, 32:64],
                                    axis=AX.X, op=OP.add)
            c0 = cp.tile([32, 1], f32)
            nc.sync.dma_start(c0[:], cvec[32:64, :])
            wv = cp.tile([32, 1], f32)
            nc.sync.dma_start(wv[:], tr_d[STOP:STOP + 1, :]
                              .rearrange("a b -> b a"))
            nc.vector.tensor_scalar_max(wv[:], wv[:], -80.0)
            nc.scalar.activation(wv[:], wv[:], AF.Exp)

            # ---------------- gold one-hot machinery ----------------
            u8 = mybir.dt.uint8
            tags_sb = cp.tile([BC, S], i32)
            nc.sync.dma_start(tags_sb[:], tg_d)
            tags8 = cp.tile([BC, S], u8)
            nc.vector.tensor_copy(tags8[:], tags_sb[:])
            iota_t = cp.tile([BC, CH * TAGSET], i32)
            nc.gpsimd.iota(iota_t[:], pattern=[[0, CH], [1, TAGSET]], base=0,
                           channel_multiplier=0)
            iota8 = cp.tile([BC, CH * TAGSET], u8)
            nc.vector.tensor_copy(iota8[:], iota_t[:])
            NACC = NCHK * NSEG
            acc_e = cp.tile([BC, NACC], f32)
            nc.vector.memset(acc_e[:], 0.0)

            # ---------------- state init ----------------
            offacc = cp.tile([64, NC], f32)
            nc.vector.memset(offacc[:], 0.0)
            ones_b = cp.tile([64, 64], bf16)
            nc.vector.memset(ones_b[:], 1.0)

            state = stp.tile([64, NC], bf16, tag="state")
            nc.vector.memset(state[:], 1.0)
            # f_1 init: one-hot START on fwd group 0
            nc.gpsimd.affine_select(
                out=state[0:32, 0:64], in_=ones_b[0:32, :], pattern=[[0, 64]],
                compare_op=OP.is_equal, fill=0.0, base=-START,
                channel_multiplier=1)

            # ---------------- chunk machinery ----------------
            # comb(c): [BC, CH*NSEG*64] f32, dims (l, j, u, t):
            #   u=0: emissions[:, j*L + c*CH + l, :]          (fwd, ascending)
            #   u=1: emissions[:, (j+1)*L - 1 - c*CH - l, :]  (bwd, descending)
            def load_chunk(c):
                comb = ccp.tile([BC, CH * NSEG * 64], f32, tag="comb")
                cv = comb[:].rearrange("b (l j u t) -> b l j u t",
                                       j=NSEG, u=2, t=TAGSET)
                hi = L - 1 - c * CH
                lo = L - (c + 1) * CH - 1
                bsl = slice(hi, None, -1) if lo < 0 else slice(hi, lo, -1)
                for j in range(NSEG):
                    nc.sync.dma_start(cv[:, :, j, 0, :],
                                      em_r[:, c * CH:(c + 1) * CH, j, :])
                    nc.sync.dma_start(cv[:, :, j, 1, :], em_r[:, bsl, j, :])
                return comb

            # per-tick xt: 8 transposes [64,64] -> one PSUM bank -> one exp
            def prep_tick(comb, l):
                trg = trp.tile([64, NC], f32, tag="trg")
                for j in range(NSEG):
                    nc.tensor.transpose(
                        trg[:, j * 64:(j + 1) * 64],
                        comb[:, (l * NSEG + j) * 64:(l * NSEG + j + 1) * 64],
                        ident[:])
                xt = xtp.tile([64, NC], bf16, tag="xt")
                nc.scalar.activation(xt[:], trg[:], AF.Exp, bias=negmu[:])
                return xt

            def gold_piece(comb, c, j):
                # one-hot gold for chunk c, segment j (fwd steps only; the
                # fwd halves of all (c, j) cover every step exactly once).
                # u8 compare -> bf16 mask, then fused multiply+reduce.
                iview = iota8[:].rearrange("b (l t) -> b l t", t=TAGSET)
                oh = ohp.tile([BC, CH * TAGSET], bf16, tag="oh")
                ov = oh[:].rearrange("b (l t) -> b l t", t=TAGSET)
                base = j * L + c * CH
                tsl = tags8[:, base:base + CH]
                tbc = tsl.rearrange("b l -> b l ()").to_broadcast(
                    [BC, CH, TAGSET])
                nc.vector.tensor_tensor(out=ov[:], in0=iview, in1=tbc,
                                        op=OP.is_equal)
                cslice = comb[:].rearrange(
                    "b (l j u t) -> b l j u t", j=NSEG, u=2,
                    t=TAGSET)[:, :, j, 0, :]
                scrap = ohp.tile([BC, CH * TAGSET], f32, tag="scrap")
                col = c * NSEG + j
                nc.vector.tensor_tensor_reduce(
                    out=scrap[:], in0=oh[:], in1=cslice, scale=1.0,
                    scalar=0.0, op0=OP.mult, op1=OP.add,
                    accum_out=acc_e[:, col:col + 1])

            # ---------------- prologue: chunk 0 xt ----------------
            comb_cur = load_chunk(0)
            xts = [prep_tick(comb_cur, l) for l in range(CH)]

            # ---------------- main scan: 256 ticks ----------------
            for tau in range(L):
                c, l = divmod(tau, CH)
                if l == 0 and c + 1 < NCHK:
                    comb_nxt = load_chunk(c + 1)
                if c + 1 < NCHK:
                    xts.append(prep_tick(comb_nxt, l))
                if gold >= 2 and l % 2 == 1:
                    gold_piece(comb_cur, c, l // 2)

                ps = mmp.tile([64, NC], f32, tag="mm")
                nc.tensor.matmul(ps[:], wb16[:], state[:], start=True,
                                 stop=True)
                nstate = stp.tile([64, NC], bf16, tag="state")
                nc.vector.tensor_mul(nstate[:], ps[:], xts[tau])
                state = nstate

                if tau == L // 2 - 1:
                    # renorm: rescale every column by its proxy-row value;
                    # only fwd log-offsets matter (h offsets cancel in the
                    # stitch's dn/dd ratio)
                    pr = smp.tile([64, NC], f32, tag="pr")
                    nc.vector.tensor_copy(pr[0:1, :], state[0:1, :])
                    nc.vector.tensor_copy(pr[32:33, :], state[32:33, :])
                    rec = smp.tile([64, NC], f32, tag="rec")
                    nc.vector.reciprocal(rec[0:1, :], pr[0:1, :])
                    nc.vector.reciprocal(rec[32:33, :], pr[32:33, :])
                    bc_ps = fip.tile([64, NC], f32, tag="bc")
                    nc.tensor.matmul(bc_ps[0:32, :], ones_t[0:1, 0:32],
                                     rec[0:1, :], start=True, stop=True)
                    nc.tensor.matmul(bc_ps[32:64, :], ones_t[32:33, 0:32],
                                     rec[32:33, :], start=True, stop=True,
                                     tile_position=(32, 32))
                    lg = smp.tile([64, NC], f32, tag="lg")
                    nc.scalar.activation(lg[0:1, :], pr[0:1, :], AF.Ln)
                    nc.vector.tensor_add(offacc[0:1, :], offacc[0:1, :],
                                         lg[0:1, :])
                    rstate = stp.tile([64, NC], bf16, tag="state")
                    nc.vector.tensor_mul(rstate[:], state[:], bc_ps[:])
                    state = rstate
                if l == CH - 1:
                    comb_cur = comb_nxt

            # ---------------- stitch ----------------
            # phi = M f (fwd-block matmul on the fwd finals)
            phi = fip.tile([32, NC], f32, tag="fin")
            nc.tensor.matmul(phi[:], wb16[0:32, 0:32], state[0:32, :],
                             start=True, stop=True)
            hs = smp.tile([32, NC], bf16, tag="hs")
            nc.sync.dma_start(hs[:], state[32:64, :])
            DN = (NSEG - 1) * 64
            pk = smp.tile([32, 2 * NC], f32, tag="pk")
            nc.vector.memset(pk[:], 1.0)
            # dn_k = h_k . (M f_{k-1}):  h groups 1..7 vs phi groups 0..6
            nc.vector.tensor_mul(pk[:, 0:DN], phi[:, 0:DN], hs[:, 64:NC])
            # dd_k = h_k . c
            nc.vector.tensor_mul(pk[:, NC:NC + DN], hs[:, 64:NC],
                                 c0[:, 0:1].to_broadcast([32, DN]))
            # d0 = w . f_m
            nc.vector.tensor_mul(pk[:, NC + DN:2 * NC],
                                 state[0:32, NC - 64:NC],
                                 wv[:, 0:1].to_broadcast([32, 64]))
            sumA = fip.tile([1, NC], f32, tag="fin")
            nc.tensor.matmul(sumA[:], ones_t[0:32, 0:1], pk[:, 0:NC],
                             start=True, stop=True)
            lnA = smp.tile([1, NC], f32, tag="lnA")
            nc.scalar.activation(lnA[:], sumA[:], AF.Ln)
            sumB = fip.tile([1, NC], f32, tag="fin")
            nc.tensor.matmul(sumB[:], ones_t[0:32, 0:1], pk[:, NC:2 * NC],
                             start=True, stop=True)
            lnB = smp.tile([1, NC], f32, tag="lnB")
            nc.scalar.activation(lnB[:], sumB[:], AF.Ln)

            lz = smp.tile([1, 64], f32, tag="lz")
            # logZ = d0ln + sum_x dn - sum_x dd + sum_k off_f + MU*S
            nc.vector.tensor_copy(lz[:], lnB[:, DN:NC])
            dnsum = smp.tile([1, 64], f32, tag="dnsum")
            nc.vector.tensor_reduce(
                dnsum[:], lnA[:, 0:DN].rearrange("p (x b) -> p b x", b=64),
                axis=AX.X, op=OP.add)
            ddsum = smp.tile([1, 64], f32, tag="ddsum")
            nc.vector.tensor_reduce(
                ddsum[:], lnB[:, 0:DN].rearrange("p (x b) -> p b x", b=64),
                axis=AX.X, op=OP.add)
            offsum = smp.tile([1, 64], f32, tag="offsum")
            nc.vector.tensor_reduce(
                offsum[:], offacc[0:1, :].rearrange("p (k b) -> p b k", b=64),
                axis=AX.X, op=OP.add)
            nc.vector.tensor_add(lz[:], lz[:], dnsum[:])
            nc.vector.tensor_sub(lz[:], lz[:], ddsum[:])
            nc.vector.tensor_add(lz[:], lz[:], offsum[:])
            # gold emission score
            gold_c = cp.tile([BC, 1], f32)
            nc.vector.tensor_reduce(gold_c[:], acc_e[:], axis=AX.X, op=OP.add)
            goldT = fip.tile([1, 64], f32, tag="fin")
            nc.tensor.transpose(goldT[0:1, :], gold_c[:, 0:1], ident[:])
            nc.vector.tensor_sub(lz[:], lz[:], goldT[0:1, :])
            nc.vector.tensor_scalar_add(lz[:], lz[:], float(MU) * S)
            nc.sync.dma_start(nll_d, lz[:])

            if _loop is not None:
                _loop.__exit__(None, None, None)

    nc.compile()
    return nc


def _get_nc():
    if "nc" not in _CACHE:
        _CACHE["nc"] = _build_nc()
    return _CACHE["nc"]


def kernel(emissions, transitions, tags):
    from concourse.bass_utils import run_bass_kernel_spmd

    em = np.ascontiguousarray(np.asarray(emissions, dtype=np.float32))
    tr = np.ascontiguousarray(np.asarray(transitions, dtype=np.float32))
    tg = np.ascontiguousarray(np.asarray(tags, dtype=np.int32))

    nc = _get_nc()
    in_maps = [
        {
            "emissions": em[c * BC:(c + 1) * BC],
            "tags": tg[c * BC:(c + 1) * BC],
            "transitions": tr,
        }
        for c in range(NCORES)
    ]
    res = run_bass_kernel_spmd(nc, in_maps, list(range(NCORES)))
    nll = np.concatenate([res.results[c]["nll"][0] for c in range(NCORES)])
    t_sc = (tr[tg[:, 1:], tg[:, :-1]].sum(axis=1)
            + tr[tg[:, 0], START] + tr[STOP, tg[:, -1]])
    total = np.sum(nll.astype(np.float64)) - np.sum(t_sc.astype(np.float64))
    return np.array(total, dtype=np.float32)
